# revision 28
# baseline (speedup 1.0000x reference)
"""Multi-head QKV block attention for Trainium2, SPMD over 8 NeuronCores.

Problem: X[4,2048,1024], residual[4,2048,1024], wq/wk/wv[1024,1024],
H=16 heads, D=64, softmax scale sqrt(S/H)=sqrt(128).
out = softmax((X wq)(X wk)^T / sqrt(128)) (X wv) + residual, returned twice.

Sharding: core c handles batch b=c//2 and head group g=c%2 (8 heads = 512
feature columns). Fully data/tensor-parallel -- no collectives; host
assembles the output. X is pre-transposed and bf16-cast on the host
(input marshaling) so the device never spends PE time transposing it.

Per-core kernel (Tile framework), fully fused single phase:
  - K/V/Q projections (bf16) run on the PE with weights/xT stationary,
    interleaved into the attention stream so the PE never drains.
  - per head pair (hp) and 512-query tile (st): transposed logits
    K_h^T.T @ Q_h^T as two K=64 matmuls in concurrent PE row groups; exp
    is computed SHIFTED (p' = exp(l*SCALE - C_SHIFT)) directly into
    fp8e5m2: ScalarE spline exp for 10 t-chunks, VectorE uint8-Schraudolph
    (tensor_scalar mult+add to uint8, bitcast e5m2 -- negative saturation
    gives a clean flush-to-zero) for 6 chunks.
  - effect^T accumulated as [v|1].T @ expT via fp8 DoubleRow matmuls
    (vS8 e4m3 stationary pairs two t-chunks per PE cell, K=256 virtual,
    ~2x the bf16 effect throughput); the ones column gives the softmax
    denominator at partition 64 of each [65,512] accumulator.
  - epilogue (deferred one head pair): ScalarE casts the accumulators to
    SBUF (freeing psum banks), PE-transposes back to [s,d], VectorE
    reciprocal + normalize, residual-add, DMA out.
  - PSUM: lp ring 3x2 banks + 2 effect-accumulator banks = 8.
  - The t-pair loop is software-pipelined (logits(p+2) before effect(p));
    kT/qt psum->SBUF copies run on VectorE.
"""

import math
import sys

for _p in ("/opt/trn_rl_repo", "/root/.axon_site/_ro/trn_rl_repo"):
    if _p not in sys.path:
        sys.path.append(_p)

import numpy as np

B, S, F = 4, 2048, 1024
H = 16
D = 64
G = 512            # feature columns per core (8 heads)
NH = 8             # heads per core
KC = 8             # contraction chunks of 128 over F
ST = 4             # s tiles of 512
TC = 16            # t chunks of 128
TP = 8             # t-chunk PAIRS (DoubleRow contracts 2 chunks at once)
SCALE = 1.0 / math.sqrt(S / H)
# exp is computed shifted by C_SHIFT: p' = exp(SCALE*l - C_SHIFT) so p' fits
# fp8e5m2 (max scaled logit 11.59 -> e^10.34 = 2^14.9 < 57344) AND the
# uint8-Schraudolph value below stays in [0, 124): no NaN encodings, and
# negative values (p' ~ 0) saturate to uint8 0 = +0.0. Numerator and
# denominator share the shift so the softmax ratio is exact.
C_SHIFT = 1.25
# Schraudolph exp2-bit-trick for DIRECT fp8e5m2 output on VectorE:
# round(l*SM + SB) as uint8 reinterpreted as e5m2 approximates
# exp(SCALE*l - C_SHIFT). 2^2 scales into the e5m2 exponent field;
# c=0.0579 centers the sawtooth (~2% rms, at the e5m2 quantization floor).
SCH_C = 0.0579
SCH_M = 4.0 * math.log2(math.e) * SCALE
SCH_B = 4.0 * (15.0 - SCH_C) - 4.0 * math.log2(math.e) * C_SHIFT
# Per-pair exp engine split: ScalarE spline exp (fp8 out) for 10 chunks,
# VectorE Schraudolph for 6; DVE chunks sit late in each pair-loop so the
# previous head pair's deferred epilogue (at pairs 0..1) never delays an
# exp the PE is about to consume.
DVE_T = frozenset((5, 7, 9, 11, 13, 15))
VD = 80            # padded per-head stride in vS8 (Ko step 640B, 16B-aligned)

_cached = None


def _build():
    import concourse.bacc as bacc
    import concourse.tile as tile
    from concourse import mybir
    from concourse.masks import make_identity

    dt = mybir.dt
    AF = mybir.ActivationFunctionType

    nc = bacc.Bacc("TRN2", target_bir_lowering=False, debug=False, num_devices=8)

    xt_d = nc.dram_tensor("xt", [128, KC, S], dt.bfloat16, kind="ExternalInput").ap()
    # Weights are host-packed [128, KC, G] (partition-major k-chunks) so each
    # matrix is ONE DMA: dma_start triggers cost ~650ns each on the issuing
    # engine, and 24 weight triggers were what delayed wv (and V-proj) by
    # ~14us at startup.
    wq_d = nc.dram_tensor("wq", [128, KC, G], dt.bfloat16, kind="ExternalInput").ap()
    wk_d = nc.dram_tensor("wk", [128, KC, G], dt.bfloat16, kind="ExternalInput").ap()
    wv_d = nc.dram_tensor("wv", [128, KC, G], dt.bfloat16, kind="ExternalInput").ap()
    res_d = nc.dram_tensor("res", [S, G], dt.bfloat16, kind="ExternalInput").ap()
    out_d = nc.dram_tensor("out", [S, G], dt.bfloat16, kind="ExternalOutput").ap()

    with tile.TileContext(nc) as tc:
        with tc.tile_pool(name="persist", bufs=1) as persist:
            identB = persist.tile([128, 128], dt.bfloat16)
            ones = persist.tile([128, NH], dt.float32)
            nbias = persist.tile([128, 1], dt.float32)
            scr = persist.tile([128, NH], dt.float32)
            xTall = persist.tile([128, KC, S], dt.bfloat16, name="xT")
            xT = [xTall[:, k, :] for k in range(KC)]
            kT = [persist.tile([128, S], dt.bfloat16, name=f"kT{m}") for m in range(4)]
            # fp8 V in DoubleRow pair layout: vS8[p][:, r, h, 0:D] holds V of
            # t-chunk 2p+r; col D is the all-ones softmax-denominator column.
            vS8 = [persist.tile([128, 2, NH, VD], dt.float8e4, name=f"vS8{p}")
                   for p in range(TP)]

            w_sb = {}
            with tc.tile_pool(name="wp", bufs=1) as wp:
                # DMA order is the consumption order of the prologue:
                #   sync:   xT(b0), wq, xT(b2), res...
                #   gpsimd: wk, xT(b1), xT(b3), wv
                # so K-proj(0,b) / Q-proj / V-proj each find their operands
                # just in time and the PE never drains waiting on X.
                def dma_w(nm, wd, eng):
                    t = wp.tile([128, KC, G], dt.bfloat16, name=f"w{nm}")
                    eng.dma_start(t[:], wd[:])
                    for k in range(KC):
                        w_sb[nm, k] = t[:, k]

                def dma_x(b, eng):
                    eng.dma_start(
                        xTall[:, :, b * 512:(b + 1) * 512],
                        xt_d[:, :, b * 512:(b + 1) * 512])

                dma_w("k", wk_d, nc.gpsimd)
                dma_x(0, nc.sync)
                dma_x(1, nc.gpsimd)
                dma_w("q", wq_d, nc.sync)
                dma_w("v", wv_d, nc.gpsimd)
                dma_x(2, nc.sync)
                dma_x(3, nc.gpsimd)

                # Init work AFTER the DMA triggers so the input transfers own
                # the HBM window from t=0; these run on otherwise-idle
                # engines while the DMAs stream.
                make_identity(nc, identB[:])
                nc.vector.memset(ones[:], 1.0)
                nc.vector.memset(nbias[:], -C_SHIFT)
                # Preload the exp table set on ScalarE while the DMAs run.
                nc.scalar.activation(scr[:], ones[:], AF.Exp)
                for p in range(TP):
                    nc.gpsimd.memset(vS8[p][:, :, :, D], 1.0)

                # PSUM: lp ring 3x2 banks (logits / proj accumulators / tp8
                # transpose scratch) + eps 2x1 banks = 8.
                with tc.tile_pool(name="pp", bufs=3, space="PSUM") as pp, \
                     tc.tile_pool(name="epp", bufs=2, space="PSUM") as epp, \
                     tc.tile_pool(name="qtsp", bufs=8) as qtsp, \
                     tc.tile_pool(name="expa", bufs=4) as expa, \
                     tc.tile_pool(name="esp", bufs=4) as esp, \
                     tc.tile_pool(name="stp", bufs=8) as stp, \
                     tc.tile_pool(name="rsp", bufs=3) as rsp, \
                     tc.tile_pool(name="rcp", bufs=4) as rcp:

                    def ptile():
                        return pp.tile([128, 1024], dt.float32, tag="lp", name="lp")

                    def emit_kproj(m, b):
                        pk = ptile()
                        for k in range(KC):
                            nc.tensor.matmul(
                                pk[:, 0:512], w_sb["k", k][:, m * 128:(m + 1) * 128],
                                xT[k][:, b * 512:(b + 1) * 512],
                                start=(k == 0), stop=(k == KC - 1))
                        nc.vector.tensor_copy(kT[m][:, b * 512:(b + 1) * 512],
                                              pk[:, 0:512])

                    def emit_vproj_j(b, j):
                        pv = ptile()
                        tci = b * 4 + j
                        for k in range(KC):
                            nc.tensor.matmul(
                                pv[:, 0:512],
                                xT[k][:, tci * 128:(tci + 1) * 128],
                                w_sb["v", k][:], start=(k == 0), stop=(k == KC - 1))
                        nc.vector.tensor_copy(
                            vS8[tci // 2][:, tci % 2, :, 0:D],
                            pv[:, 0:512].rearrange("p (h d) -> p h d", h=NH))

                    def emit_qproj(dst_st, m):
                        pq = ptile()
                        for k in range(KC):
                            nc.tensor.matmul(
                                pq[:, 0:512], w_sb["q", k][:, m * 128:(m + 1) * 128],
                                xT[k][:, dst_st * 512:(dst_st + 1) * 512],
                                start=(k == 0), stop=(k == KC - 1))
                        qt = qtsp.tile([128, 512], dt.bfloat16, tag="qts", name="qt")
                        nc.vector.tensor_copy(qt[:], pq[:, 0:512])
                        return qt

                    def emit_chunk(hp, qts, t, ex8, r):
                        # one 2-bank psum tile holds both halves' logits for
                        # this t-chunk: the two K=64 matmuls run concurrently
                        # in disjoint PE row groups. exp (shifted by C_SHIFT)
                        # lands in fp8 slot r of the pair tile ex8.
                        lp = ptile()
                        for half in range(2):
                            r0 = half * 64
                            nc.tensor.matmul(
                                lp[:, half * 512:(half + 1) * 512],
                                kT[hp][r0:r0 + 64, t * 128:(t + 1) * 128],
                                qts[r0:r0 + 64, :],
                                start=True, stop=True)
                        # exp: alternate engines so neither paces the loop.
                        if t in DVE_T:
                            nc.vector.tensor_scalar(
                                ex8[:, r, :].bitcast(dt.uint8), lp[:], SCH_M, SCH_B,
                                mybir.AluOpType.mult, mybir.AluOpType.add)
                        elif t < 5:
                            # pairs 0-1 (and chunk 4) have no DVE chunk;
                            # split these chunks by column across both engines
                            # so the serial ScalarE stretch (which gates the
                            # lp ring) is halved at every head-pair boundary.
                            nc.scalar.activation(ex8[:, r, 0:512], lp[:, 0:512],
                                                 AF.Exp, scale=SCALE,
                                                 bias=nbias[:])
                            nc.vector.tensor_scalar(
                                ex8[:, r, 512:1024].bitcast(dt.uint8),
                                lp[:, 512:1024], SCH_M, SCH_B,
                                mybir.AluOpType.mult, mybir.AluOpType.add)
                        else:
                            nc.scalar.activation(ex8[:, r, :], lp[:], AF.Exp,
                                                 scale=SCALE, bias=nbias[:])

                    def emit_pair(hp, qts, p):
                        ex8 = expa.tile([128, 2, 1024], dt.float8e5,
                                        tag="ex8", name="ex8")
                        emit_chunk(hp, qts, 2 * p, ex8, 0)
                        emit_chunk(hp, qts, 2 * p + 1, ex8, 1)
                        return ex8

                    # ---- PE warm-up spin: ~4.5us of dependency-free matmuls
                    # (identB @ identB) so the HAM activity window fills and
                    # the clock gate opens to 2.4 GHz BEFORE the real
                    # projections start. Without it the DMA-wait gaps in the
                    # prologue keep resetting the window and the first ~30us
                    # run at half clock. (Transpose-mode would not work here:
                    # it does not count as PE-busy for HAM.)
                    wu = ptile()
                    for _ in range(40):
                        nc.tensor.matmul(wu[:, 0:128], identB[:], identB[:],
                                         start=True, stop=True)

                    # ---- prologue, in DMA-arrival order: K-proj(0,b0) and
                    # Q-proj st=0 only need x-block b0 (+wk/wq, first on
                    # their queues); later K-proj blocks consume b1..b3 as
                    # they land; V-proj last (wv is the last weight DMA).
                    emit_kproj(0, 0)
                    qts_cur = [emit_qproj(0, m) for m in range(4)]
                    emit_kproj(0, 1)
                    emit_kproj(0, 2)
                    for j in range(4):
                        emit_vproj_j(0, j)
                    emit_kproj(0, 3)

                    # Remaining projections injected into the st=0 pair-loops,
                    # each emitted before its first consumer: vproj for t-chunks
                    # (2p+4, 2p+5) at pair-slot p (consumed at slot p+2 of the
                    # SAME hp=0 loop), kproj(m,*) anywhere before C(0,m) starts.
                    inject = {
                        (0, 0): dict(
                            [(p, [lambda b=b, j=j: emit_vproj_j(b, j)
                                  for (b, j) in (((2 * p + 4) // 4, (2 * p + 4) % 4),
                                                 ((2 * p + 5) // 4, (2 * p + 5) % 4))])
                             for p in range(6)] +
                            [(6, [lambda: emit_kproj(1, 0), lambda: emit_kproj(1, 1)]),
                             (7, [lambda: emit_kproj(1, 2), lambda: emit_kproj(1, 3)])]),
                        (0, 1): {1 + 2 * b: [lambda b=b: emit_kproj(2, b)]
                                 for b in range(4)},
                        (0, 2): {1 + 2 * b: [lambda b=b: emit_kproj(3, b)]
                                 for b in range(4)},
                    }

                    def make_epilogue(hp, ess, stage, fin=None, last=False):
                        # deferred epilogue tail (all VectorE, so cross-engine
                        # waits never sit at the head of ScalarE's exp FIFO):
                        # PE-transpose both halves into a psum-ring scratch
                        # (bitcast view), normalize by the ones-row
                        # denominator, write into stage. The effect^T bf16
                        # casts are NOT here -- they run at the owning
                        # iteration's end so the eps accumulator banks are
                        # free before the next iteration's effect(0). When
                        # this is hp=3's epilogue and the st-finish is
                        # pending, the residual-add + output DMA are
                        # interleaved j-major so the final DMAs start as
                        # early as possible.
                        def run():
                            tp8 = ptile()[:, 0:264].bitcast(dt.bfloat16) \
                                .rearrange("p (h c) -> p h c", c=D + 2)
                            for half in range(2):
                                for j in range(4):
                                    nc.tensor.transpose(
                                        tp8[:, half * 4 + j, 0:D + 1],
                                        ess[half][:, j * 128:(j + 1) * 128],
                                        identB[0:D + 1, 0:D + 1])
                            rec = rcp.tile([128, 8], dt.float32, tag="rec", name="rec")
                            nc.vector.reciprocal(rec[:], tp8[:, :, D])
                            for j in range(4):
                                for half in range(2):
                                    h = 2 * hp + half
                                    idx = half * 4 + j
                                    if last and half == 0:
                                        nc.scalar.activation(
                                            stage[j][:, h * 64:(h + 1) * 64],
                                            tp8[:, idx, 0:D], AF.Copy,
                                            scale=rec[:, idx:idx + 1])
                                    else:
                                        nc.vector.tensor_scalar_mul(
                                            stage[j][:, h * 64:(h + 1) * 64],
                                            tp8[:, idx, 0:D],
                                            rec[:, idx:idx + 1])
                                if fin is not None:
                                    s0, rts = fin
                                    nc.vector.tensor_add(stage[j][:], stage[j][:],
                                                         rts[j][:])
                                    nc.sync.dma_start(
                                        out_d[s0 + j * 128:s0 + (j + 1) * 128, :],
                                        stage[j][:])
                        return run

                    # The per-head-pair epilogue (and, for hp=3, the per-st
                    # residual+DMA) is deferred into the NEXT iteration's
                    # t-loop (t=0) so iteration boundaries never serialize
                    # the logits->exp->effect pipeline.
                    pend_epi = None
                    qts_next = [None] * 4
                    for st in range(ST):
                        s0 = st * 512
                        # Prefetch this tile's residual rows early (one DMA).
                        rtt = rsp.tile([128, 4, G], dt.bfloat16, tag="res", name="rt")
                        nc.sync.dma_start(
                            rtt[:], res_d[s0:s0 + 512, :]
                            .rearrange("(j p) g -> p j g", p=128))
                        rts = [rtt[:, j, :] for j in range(4)]
                        stage = [stp.tile([128, G], dt.bfloat16, tag="stage", name="stage")
                                 for _ in range(4)]
                        for hp in range(4):
                            inj = inject.get((st, hp), {})
                            eps = [epp.tile([D + 1, 512], dt.float32, tag="ep", name="ep")
                                   for _ in range(2)]
                            # software pipeline depth 2 pairs: logits(p+2) is
                            # emitted before effect(p) so the PE always has
                            # queued work to cover the exp latency.
                            exq = [emit_pair(hp, qts_cur[hp], 0),
                                   emit_pair(hp, qts_cur[hp], 1)]
                            for p in range(TP):
                                for fn in inj.get(p, ()):
                                    fn()
                                if p < TP - 2:
                                    exq.append(emit_pair(hp, qts_cur[hp], p + 2))
                                if p == 0 and pend_epi is not None:
                                    pend_epi()
                                    pend_epi = None
                                if p == 4 and st < ST - 1:
                                    qts_next[hp] = emit_qproj(st + 1, hp)
                                ex8 = exq.pop(0)
                                # effect^T: one DoubleRow matmul per head
                                # contracts BOTH t-chunks of the pair (2 fp8
                                # weights per PE cell, K=256 virtual).
                                for half in range(2):
                                    nc.tensor.matmul(
                                        eps[half][:],
                                        vS8[p][:, :, 2 * hp + half, 0:D + 1],
                                        ex8[:, :, half * 512:(half + 1) * 512],
                                        start=(p == 0), stop=(p == TP - 1),
                                        perf_mode=mybir.MatmulPerfMode.DoubleRow)
                            # cast effect^T to bf16 NOW (frees the eps psum
                            # banks for the next iteration's effect(0)); the
                            # last iteration splits the casts across ScalarE
                            # and VectorE since nothing else is in flight.
                            last = st == ST - 1 and hp == 3
                            ess = []
                            for half in range(2):
                                es = esp.tile([D + 1, 512], dt.bfloat16,
                                              tag="es", name="es")
                                if last and half == 0:
                                    nc.scalar.copy(es[:], eps[half][:])
                                else:
                                    nc.vector.tensor_copy(es[:], eps[half][:])
                                ess.append(es)
                            pend_epi = make_epilogue(
                                hp, ess, stage,
                                fin=(s0, rts) if hp == 3 else None,
                                last=last)
                        qts_cur = qts_next
                        qts_next = [None] * 4
                    pend_epi()

    nc.compile()
    return nc


def _get_nc():
    global _cached
    if _cached is None:
        _cached = _build()
    return _cached


def _make_in_maps(X, residual_score, wq, wk, wv):
    import ml_dtypes

    bf16 = ml_dtypes.bfloat16

    def pack(a):
        # [F, cols] -> [128, KC, cols] partition-major k-chunks (one DMA).
        return np.ascontiguousarray(
            a.reshape(KC, 128, a.shape[1]).transpose(1, 0, 2))

    X = np.asarray(X, dtype=np.float32)
    residual_score = np.asarray(residual_score, dtype=np.float32).astype(bf16)
    wq = np.asarray(wq, dtype=np.float32).astype(bf16)
    wk = np.asarray(wk, dtype=np.float32).astype(bf16)
    wv = np.asarray(wv, dtype=np.float32).astype(bf16)
    xts = [pack(X[b].T.astype(bf16)) for b in range(B)]
    in_maps = []
    for c in range(8):
        b, g = c // 2, c % 2
        cols = slice(g * G, (g + 1) * G)
        in_maps.append({
            "xt": xts[b],
            "wq": pack(wq[:, cols]),
            "wk": pack(wk[:, cols]),
            "wv": pack(wv[:, cols]),
            "res": np.ascontiguousarray(residual_score[b, :, cols]),
        })
    return in_maps


def _assemble(results):
    out = np.empty((B, S, F), dtype=np.float32)
    for c in range(8):
        b, g = c // 2, c % 2
        out[b, :, g * G:(g + 1) * G] = np.asarray(results[c]["out"],
                                                  dtype=np.float32)
    return out


def run(X, residual_score, wq, wk, wv, trace=False):
    from concourse.bass_utils import run_bass_kernel_spmd

    nc = _get_nc()
    in_maps = _make_in_maps(X, residual_score, wq, wk, wv)
    res = run_bass_kernel_spmd(nc, in_maps, core_ids=list(range(8)), trace=trace)
    return _assemble(res.results), res


def kernel(X, residual_score, wq, wk, wv):
    out, _ = run(X, residual_score, wq, wk, wv)
    return (out, out)



# revision 30
# speedup vs baseline: 1.0155x; 1.0155x over previous
"""Multi-head QKV block attention for Trainium2, SPMD over 8 NeuronCores.

Problem: X[4,2048,1024], residual[4,2048,1024], wq/wk/wv[1024,1024],
H=16 heads, D=64, softmax scale sqrt(S/H)=sqrt(128).
out = softmax((X wq)(X wk)^T / sqrt(128)) (X wv) + residual, returned twice.

Sharding: core c handles batch b=c//2 and head group g=c%2 (8 heads = 512
feature columns). Fully data/tensor-parallel -- no collectives; host
assembles the output. X is pre-transposed and bf16-cast on the host
(input marshaling) so the device never spends PE time transposing it.

Per-core kernel (Tile framework), fully fused single phase:
  - K/V/Q projections (bf16) run on the PE with weights/xT stationary,
    interleaved into the attention stream so the PE never drains.
  - per head pair (hp) and 512-query tile (st): transposed logits
    K_h^T.T @ Q_h^T as two K=64 matmuls in concurrent PE row groups; exp
    is computed SHIFTED (p' = exp(l*SCALE - C_SHIFT)) directly into
    fp8e5m2: ScalarE spline exp for 10 t-chunks, VectorE uint8-Schraudolph
    (tensor_scalar mult+add to uint8, bitcast e5m2 -- negative saturation
    gives a clean flush-to-zero) for 6 chunks.
  - effect^T accumulated as [v|1].T @ expT via fp8 DoubleRow matmuls
    (vS8 e4m3 stationary pairs two t-chunks per PE cell, K=256 virtual,
    ~2x the bf16 effect throughput); the ones column gives the softmax
    denominator at partition 64 of each [65,512] accumulator.
  - epilogue (deferred one head pair): ScalarE casts the accumulators to
    SBUF (freeing psum banks), PE-transposes back to [s,d], VectorE
    reciprocal + normalize, residual-add, DMA out.
  - PSUM: lp ring 3x2 banks + 2 effect-accumulator banks = 8.
  - The t-pair loop is software-pipelined (logits(p+2) before effect(p));
    kT/qt psum->SBUF copies run on VectorE.
"""

import math
import sys

for _p in ("/opt/trn_rl_repo", "/root/.axon_site/_ro/trn_rl_repo"):
    if _p not in sys.path:
        sys.path.append(_p)

import numpy as np

B, S, F = 4, 2048, 1024
H = 16
D = 64
G = 512            # feature columns per core (8 heads)
NH = 8             # heads per core
KC = 8             # contraction chunks of 128 over F
ST = 4             # s tiles of 512
TC = 16            # t chunks of 128
TP = 8             # t-chunk PAIRS (DoubleRow contracts 2 chunks at once)
SCALE = 1.0 / math.sqrt(S / H)
# exp is computed shifted by C_SHIFT: p' = exp(SCALE*l - C_SHIFT) so p' fits
# fp8e5m2 (max scaled logit 11.59 -> e^10.34 = 2^14.9 < 57344) AND the
# uint8-Schraudolph value below stays in [0, 124): no NaN encodings, and
# negative values (p' ~ 0) saturate to uint8 0 = +0.0. Numerator and
# denominator share the shift so the softmax ratio is exact.
C_SHIFT = 1.25
# Schraudolph exp2-bit-trick for DIRECT fp8e5m2 output on VectorE:
# round(l*SM + SB) as uint8 reinterpreted as e5m2 approximates
# exp(SCALE*l - C_SHIFT). 2^2 scales into the e5m2 exponent field;
# c=0.0579 centers the sawtooth (~2% rms, at the e5m2 quantization floor).
SCH_C = 0.0579
SCH_M = 4.0 * math.log2(math.e) * SCALE
SCH_B = 4.0 * (15.0 - SCH_C) - 4.0 * math.log2(math.e) * C_SHIFT
# Per-pair exp engine split: ScalarE spline exp (fp8 out) for 10 chunks,
# VectorE Schraudolph for 6; DVE chunks sit late in each pair-loop so the
# previous head pair's deferred epilogue (at pairs 0..1) never delays an
# exp the PE is about to consume.
DVE_T = frozenset((5, 7, 9, 11, 13, 15))
VD = 80            # padded per-head stride in vS8 (Ko step 640B, 16B-aligned)

_cached = None


def _build():
    import concourse.bacc as bacc
    import concourse.tile as tile
    from concourse import mybir
    from concourse.masks import make_identity

    dt = mybir.dt
    AF = mybir.ActivationFunctionType

    nc = bacc.Bacc("TRN2", target_bir_lowering=False, debug=False, num_devices=8)

    xt_d = nc.dram_tensor("xt", [128, KC, S], dt.bfloat16, kind="ExternalInput").ap()
    # Weights are host-packed [128, KC, G] (partition-major k-chunks) so each
    # matrix is ONE DMA: dma_start triggers cost ~650ns each on the issuing
    # engine, and 24 weight triggers were what delayed wv (and V-proj) by
    # ~14us at startup.
    wq_d = nc.dram_tensor("wq", [128, KC, G], dt.bfloat16, kind="ExternalInput").ap()
    wk_d = nc.dram_tensor("wk", [128, KC, G], dt.bfloat16, kind="ExternalInput").ap()
    wv_d = nc.dram_tensor("wv", [128, KC, G], dt.bfloat16, kind="ExternalInput").ap()
    res_d = nc.dram_tensor("res", [S, G], dt.bfloat16, kind="ExternalInput").ap()
    out_d = nc.dram_tensor("out", [S, G], dt.bfloat16, kind="ExternalOutput").ap()

    with tile.TileContext(nc) as tc:
        with tc.tile_pool(name="persist", bufs=1) as persist:
            identB = persist.tile([128, 128], dt.bfloat16)
            ones = persist.tile([128, NH], dt.float32)
            nbias = persist.tile([128, 1], dt.float32)
            scr = persist.tile([128, NH], dt.float32)
            xTall = persist.tile([128, KC, S], dt.bfloat16, name="xT")
            xT = [xTall[:, k, :] for k in range(KC)]
            kT = [persist.tile([128, S], dt.bfloat16, name=f"kT{m}") for m in range(4)]
            # fp8 V in DoubleRow pair layout: vS8[p][:, r, h, 0:D] holds V of
            # t-chunk 2p+r; col D is the all-ones softmax-denominator column.
            vS8 = [persist.tile([128, 2, NH, VD], dt.float8e4, name=f"vS8{p}")
                   for p in range(TP)]

            w_sb = {}
            with tc.tile_pool(name="wp", bufs=1) as wp:
                # DMA order is the consumption order of the prologue:
                #   sync:   xT(b0), wq, xT(b2), res...
                #   gpsimd: wk, xT(b1), xT(b3), wv
                # so K-proj(0,b) / Q-proj / V-proj each find their operands
                # just in time and the PE never drains waiting on X.
                def dma_w(nm, wd, eng):
                    t = wp.tile([128, KC, G], dt.bfloat16, name=f"w{nm}")
                    eng.dma_start(t[:], wd[:])
                    for k in range(KC):
                        w_sb[nm, k] = t[:, k]

                def dma_x(b, eng):
                    eng.dma_start(
                        xTall[:, :, b * 512:(b + 1) * 512],
                        xt_d[:, :, b * 512:(b + 1) * 512])

                # wk and x-block0 gate the first K-proj: split each into
                # halves so the k-loop starts as soon as k-chunks 0-3 land.
                wk_t = wp.tile([128, KC, G], dt.bfloat16, name="wk")
                nc.gpsimd.dma_start(wk_t[:, 0:4], wk_d[:, 0:4])
                nc.sync.dma_start(xTall[:, 0:4, 0:512], xt_d[:, 0:4, 0:512])
                # identB right after the critical triggers: the PE warm-up
                # spin depends only on it and should start ~1.5us in.
                make_identity(nc, identB[:])
                nc.gpsimd.dma_start(wk_t[:, 4:8], wk_d[:, 4:8])
                nc.sync.dma_start(xTall[:, 4:8, 0:512], xt_d[:, 4:8, 0:512])
                for k in range(KC):
                    w_sb["k", k] = wk_t[:, k]
                dma_x(1, nc.gpsimd)
                dma_w("q", wq_d, nc.sync)
                dma_w("v", wv_d, nc.gpsimd)
                dma_x(2, nc.sync)
                dma_x(3, nc.gpsimd)

                # Remaining init on otherwise-idle engines while DMAs stream.
                nc.vector.memset(ones[:], 1.0)
                nc.vector.memset(nbias[:], -C_SHIFT)
                # Preload the exp table set on ScalarE while the DMAs run.
                nc.scalar.activation(scr[:], ones[:], AF.Exp)
                for p in range(TP):
                    nc.gpsimd.memset(vS8[p][:, :, :, D], 1.0)

                # PSUM: lp ring 3x2 banks (logits / proj accumulators / tp8
                # transpose scratch) + eps 2x1 banks = 8.
                with tc.tile_pool(name="pp", bufs=3, space="PSUM") as pp, \
                     tc.tile_pool(name="epp", bufs=2, space="PSUM") as epp, \
                     tc.tile_pool(name="qtsp", bufs=8) as qtsp, \
                     tc.tile_pool(name="expa", bufs=4) as expa, \
                     tc.tile_pool(name="esp", bufs=4) as esp, \
                     tc.tile_pool(name="stp", bufs=8) as stp, \
                     tc.tile_pool(name="rsp", bufs=3) as rsp, \
                     tc.tile_pool(name="rcp", bufs=4) as rcp:

                    def ptile():
                        return pp.tile([128, 1024], dt.float32, tag="lp", name="lp")

                    def emit_kproj(m, b):
                        pk = ptile()
                        for k in range(KC):
                            nc.tensor.matmul(
                                pk[:, 0:512], w_sb["k", k][:, m * 128:(m + 1) * 128],
                                xT[k][:, b * 512:(b + 1) * 512],
                                start=(k == 0), stop=(k == KC - 1))
                        nc.vector.tensor_copy(kT[m][:, b * 512:(b + 1) * 512],
                                              pk[:, 0:512])

                    def emit_vproj_j(b, j):
                        pv = ptile()
                        tci = b * 4 + j
                        for k in range(KC):
                            nc.tensor.matmul(
                                pv[:, 0:512],
                                xT[k][:, tci * 128:(tci + 1) * 128],
                                w_sb["v", k][:], start=(k == 0), stop=(k == KC - 1))
                        nc.vector.tensor_copy(
                            vS8[tci // 2][:, tci % 2, :, 0:D],
                            pv[:, 0:512].rearrange("p (h d) -> p h d", h=NH))

                    def emit_qproj(dst_st, m):
                        pq = ptile()
                        for k in range(KC):
                            nc.tensor.matmul(
                                pq[:, 0:512], w_sb["q", k][:, m * 128:(m + 1) * 128],
                                xT[k][:, dst_st * 512:(dst_st + 1) * 512],
                                start=(k == 0), stop=(k == KC - 1))
                        qt = qtsp.tile([128, 512], dt.bfloat16, tag="qts", name="qt")
                        nc.vector.tensor_copy(qt[:], pq[:, 0:512])
                        return qt

                    def emit_chunk(hp, qts, t, ex8, r):
                        # one 2-bank psum tile holds both halves' logits for
                        # this t-chunk: the two K=64 matmuls run concurrently
                        # in disjoint PE row groups. exp (shifted by C_SHIFT)
                        # lands in fp8 slot r of the pair tile ex8.
                        lp = ptile()
                        for half in range(2):
                            r0 = half * 64
                            nc.tensor.matmul(
                                lp[:, half * 512:(half + 1) * 512],
                                kT[hp][r0:r0 + 64, t * 128:(t + 1) * 128],
                                qts[r0:r0 + 64, :],
                                start=True, stop=True)
                        # exp: alternate engines so neither paces the loop.
                        if t in DVE_T:
                            nc.vector.tensor_scalar(
                                ex8[:, r, :].bitcast(dt.uint8), lp[:], SCH_M, SCH_B,
                                mybir.AluOpType.mult, mybir.AluOpType.add)
                        elif t < 4:
                            # pairs 0-1 have no DVE chunk; split these chunks
                            # by column across both engines so the serial
                            # ScalarE stretch (which gates the lp ring) is
                            # halved at every head-pair boundary.
                            nc.scalar.activation(ex8[:, r, 0:512], lp[:, 0:512],
                                                 AF.Exp, scale=SCALE,
                                                 bias=nbias[:])
                            nc.vector.tensor_scalar(
                                ex8[:, r, 512:1024].bitcast(dt.uint8),
                                lp[:, 512:1024], SCH_M, SCH_B,
                                mybir.AluOpType.mult, mybir.AluOpType.add)
                        else:
                            nc.scalar.activation(ex8[:, r, :], lp[:], AF.Exp,
                                                 scale=SCALE, bias=nbias[:])

                    def emit_pair(hp, qts, p):
                        ex8 = expa.tile([128, 2, 1024], dt.float8e5,
                                        tag="ex8", name="ex8")
                        emit_chunk(hp, qts, 2 * p, ex8, 0)
                        emit_chunk(hp, qts, 2 * p + 1, ex8, 1)
                        return ex8

                    # ---- PE warm-up spin: ~4.5us of dependency-free matmuls
                    # (identB @ identB) so the HAM activity window fills and
                    # the clock gate opens to 2.4 GHz BEFORE the real
                    # projections start. Without it the DMA-wait gaps in the
                    # prologue keep resetting the window and the first ~30us
                    # run at half clock. (Transpose-mode would not work here:
                    # it does not count as PE-busy for HAM.)
                    wu = ptile()
                    for _ in range(40):
                        nc.tensor.matmul(wu[:, 0:128], identB[:], identB[:],
                                         start=True, stop=True)

                    # ---- prologue, in DMA-arrival order: K-proj(0,b0) and
                    # Q-proj st=0 only need x-block b0 (+wk/wq, first on
                    # their queues); later K-proj blocks consume b1..b3 as
                    # they land; V-proj last (wv is the last weight DMA).
                    emit_kproj(0, 0)
                    qts_cur = [emit_qproj(0, m) for m in range(4)]
                    emit_kproj(0, 1)
                    emit_kproj(0, 2)
                    for j in range(4):
                        emit_vproj_j(0, j)
                    emit_kproj(0, 3)

                    # Remaining projections injected into the st=0 pair-loops,
                    # each emitted before its first consumer: vproj for t-chunks
                    # (2p+4, 2p+5) at pair-slot p (consumed at slot p+2 of the
                    # SAME hp=0 loop), kproj(m,*) anywhere before C(0,m) starts.
                    inject = {
                        (0, 0): dict(
                            [(p, [lambda b=b, j=j: emit_vproj_j(b, j)
                                  for (b, j) in (((2 * p + 4) // 4, (2 * p + 4) % 4),
                                                 ((2 * p + 5) // 4, (2 * p + 5) % 4))])
                             for p in range(6)] +
                            [(6, [lambda: emit_kproj(1, 0), lambda: emit_kproj(1, 1)]),
                             (7, [lambda: emit_kproj(1, 2), lambda: emit_kproj(1, 3)])]),
                        (0, 1): {1 + 2 * b: [lambda b=b: emit_kproj(2, b)]
                                 for b in range(4)},
                        (0, 2): {1 + 2 * b: [lambda b=b: emit_kproj(3, b)]
                                 for b in range(4)},
                    }

                    def make_epilogue(hp, ess, stage, fin=None, last=False):
                        # deferred epilogue tail (all VectorE, so cross-engine
                        # waits never sit at the head of ScalarE's exp FIFO):
                        # PE-transpose both halves into a psum-ring scratch
                        # (bitcast view), normalize by the ones-row
                        # denominator, write into stage. The effect^T bf16
                        # casts are NOT here -- they run at the owning
                        # iteration's end so the eps accumulator banks are
                        # free before the next iteration's effect(0). When
                        # this is hp=3's epilogue and the st-finish is
                        # pending, the residual-add + output DMA are
                        # interleaved j-major so the final DMAs start as
                        # early as possible.
                        def run():
                            tp8 = ptile()[:, 0:264].bitcast(dt.bfloat16) \
                                .rearrange("p (h c) -> p h c", c=D + 2)
                            for half in range(2):
                                for j in range(4):
                                    nc.tensor.transpose(
                                        tp8[:, half * 4 + j, 0:D + 1],
                                        ess[half][:, j * 128:(j + 1) * 128],
                                        identB[0:D + 1, 0:D + 1])
                            rec = rcp.tile([128, 8], dt.float32, tag="rec", name="rec")
                            nc.vector.reciprocal(rec[:], tp8[:, :, D])
                            for j in range(4):
                                for half in range(2):
                                    h = 2 * hp + half
                                    idx = half * 4 + j
                                    if last and half == 0:
                                        nc.scalar.activation(
                                            stage[j][:, h * 64:(h + 1) * 64],
                                            tp8[:, idx, 0:D], AF.Copy,
                                            scale=rec[:, idx:idx + 1])
                                    else:
                                        nc.vector.tensor_scalar_mul(
                                            stage[j][:, h * 64:(h + 1) * 64],
                                            tp8[:, idx, 0:D],
                                            rec[:, idx:idx + 1])
                                if fin is not None:
                                    s0, rts = fin
                                    nc.vector.tensor_add(stage[j][:], stage[j][:],
                                                         rts[j][:])
                                    nc.sync.dma_start(
                                        out_d[s0 + j * 128:s0 + (j + 1) * 128, :],
                                        stage[j][:])
                        return run

                    # The per-head-pair epilogue (and, for hp=3, the per-st
                    # residual+DMA) is deferred into the NEXT iteration's
                    # t-loop (t=0) so iteration boundaries never serialize
                    # the logits->exp->effect pipeline.
                    pend_epi = None
                    qts_next = [None] * 4
                    for st in range(ST):
                        s0 = st * 512
                        # Prefetch this tile's residual rows early (one DMA).
                        rtt = rsp.tile([128, 4, G], dt.bfloat16, tag="res", name="rt")
                        nc.sync.dma_start(
                            rtt[:], res_d[s0:s0 + 512, :]
                            .rearrange("(j p) g -> p j g", p=128))
                        rts = [rtt[:, j, :] for j in range(4)]
                        stage = [stp.tile([128, G], dt.bfloat16, tag="stage", name="stage")
                                 for _ in range(4)]
                        for hp in range(4):
                            inj = inject.get((st, hp), {})
                            eps = [epp.tile([D + 1, 512], dt.float32, tag="ep", name="ep")
                                   for _ in range(2)]
                            # software pipeline depth 2 pairs: logits(p+2) is
                            # emitted before effect(p) so the PE always has
                            # queued work to cover the exp latency.
                            exq = [emit_pair(hp, qts_cur[hp], 0),
                                   emit_pair(hp, qts_cur[hp], 1)]
                            for p in range(TP):
                                for fn in inj.get(p, ()):
                                    fn()
                                if p < TP - 2:
                                    exq.append(emit_pair(hp, qts_cur[hp], p + 2))
                                if p == 0 and pend_epi is not None:
                                    pend_epi()
                                    pend_epi = None
                                if p == 4 and st < ST - 1:
                                    qts_next[hp] = emit_qproj(st + 1, hp)
                                ex8 = exq.pop(0)
                                # effect^T: one DoubleRow matmul per head
                                # contracts BOTH t-chunks of the pair (2 fp8
                                # weights per PE cell, K=256 virtual).
                                for half in range(2):
                                    nc.tensor.matmul(
                                        eps[half][:],
                                        vS8[p][:, :, 2 * hp + half, 0:D + 1],
                                        ex8[:, :, half * 512:(half + 1) * 512],
                                        start=(p == 0), stop=(p == TP - 1),
                                        perf_mode=mybir.MatmulPerfMode.DoubleRow)
                            # cast effect^T to bf16 NOW (frees the eps psum
                            # banks for the next iteration's effect(0)); the
                            # last iteration splits the casts across ScalarE
                            # and VectorE since nothing else is in flight.
                            last = st == ST - 1 and hp == 3
                            ess = []
                            for half in range(2):
                                es = esp.tile([D + 1, 512], dt.bfloat16,
                                              tag="es", name="es")
                                if last and half == 0:
                                    nc.scalar.copy(es[:], eps[half][:])
                                else:
                                    nc.vector.tensor_copy(es[:], eps[half][:])
                                ess.append(es)
                            pend_epi = make_epilogue(
                                hp, ess, stage,
                                fin=(s0, rts) if hp == 3 else None,
                                last=last)
                        qts_cur = qts_next
                        qts_next = [None] * 4
                    pend_epi()

    nc.compile()
    return nc


def _get_nc():
    global _cached
    if _cached is None:
        _cached = _build()
    return _cached


def _make_in_maps(X, residual_score, wq, wk, wv):
    import ml_dtypes

    bf16 = ml_dtypes.bfloat16

    def pack(a):
        # [F, cols] -> [128, KC, cols] partition-major k-chunks (one DMA).
        return np.ascontiguousarray(
            a.reshape(KC, 128, a.shape[1]).transpose(1, 0, 2))

    X = np.asarray(X, dtype=np.float32)
    residual_score = np.asarray(residual_score, dtype=np.float32).astype(bf16)
    wq = np.asarray(wq, dtype=np.float32).astype(bf16)
    wk = np.asarray(wk, dtype=np.float32).astype(bf16)
    wv = np.asarray(wv, dtype=np.float32).astype(bf16)
    xts = [pack(X[b].T.astype(bf16)) for b in range(B)]
    in_maps = []
    for c in range(8):
        b, g = c // 2, c % 2
        cols = slice(g * G, (g + 1) * G)
        in_maps.append({
            "xt": xts[b],
            "wq": pack(wq[:, cols]),
            "wk": pack(wk[:, cols]),
            "wv": pack(wv[:, cols]),
            "res": np.ascontiguousarray(residual_score[b, :, cols]),
        })
    return in_maps


def _assemble(results):
    out = np.empty((B, S, F), dtype=np.float32)
    for c in range(8):
        b, g = c // 2, c % 2
        out[b, :, g * G:(g + 1) * G] = np.asarray(results[c]["out"],
                                                  dtype=np.float32)
    return out


def run(X, residual_score, wq, wk, wv, trace=False):
    from concourse.bass_utils import run_bass_kernel_spmd

    nc = _get_nc()
    in_maps = _make_in_maps(X, residual_score, wq, wk, wv)
    res = run_bass_kernel_spmd(nc, in_maps, core_ids=list(range(8)), trace=trace)
    return _assemble(res.results), res


def kernel(X, residual_score, wq, wk, wv):
    out, _ = run(X, residual_score, wq, wk, wv)
    return (out, out)



# revision 31
# speedup vs baseline: 1.0368x; 1.0210x over previous
"""Multi-head QKV block attention for Trainium2, SPMD over 8 NeuronCores.

Problem: X[4,2048,1024], residual[4,2048,1024], wq/wk/wv[1024,1024],
H=16 heads, D=64, softmax scale sqrt(S/H)=sqrt(128).
out = softmax((X wq)(X wk)^T / sqrt(128)) (X wv) + residual, returned twice.

Sharding: core c handles batch b=c//2 and head group g=c%2 (8 heads = 512
feature columns). Fully data/tensor-parallel -- no collectives; host
assembles the output. X is pre-transposed and bf16-cast on the host
(input marshaling) so the device never spends PE time transposing it.

Per-core kernel (Tile framework), fully fused single phase:
  - K/V/Q projections (bf16) run on the PE with weights/xT stationary,
    interleaved into the attention stream so the PE never drains.
  - per head pair (hp) and 512-query tile (st): transposed logits
    K_h^T.T @ Q_h^T as two K=64 matmuls in concurrent PE row groups; exp
    is computed SHIFTED (p' = exp(l*SCALE - C_SHIFT)) directly into
    fp8e5m2: ScalarE spline exp for 10 t-chunks, VectorE uint8-Schraudolph
    (tensor_scalar mult+add to uint8, bitcast e5m2 -- negative saturation
    gives a clean flush-to-zero) for 6 chunks.
  - effect^T accumulated as [v|1].T @ expT via fp8 DoubleRow matmuls
    (vS8 e4m3 stationary pairs two t-chunks per PE cell, K=256 virtual,
    ~2x the bf16 effect throughput); the ones column gives the softmax
    denominator at partition 64 of each [65,512] accumulator.
  - epilogue (deferred one head pair): ScalarE casts the accumulators to
    SBUF (freeing psum banks), PE-transposes back to [s,d], VectorE
    reciprocal + normalize, residual-add, DMA out.
  - PSUM: lp ring 3x2 banks + 2 effect-accumulator banks = 8.
  - The t-pair loop is software-pipelined (logits(p+2) before effect(p));
    kT/qt psum->SBUF copies run on VectorE.
"""

import math
import sys

for _p in ("/opt/trn_rl_repo", "/root/.axon_site/_ro/trn_rl_repo"):
    if _p not in sys.path:
        sys.path.append(_p)

import numpy as np

B, S, F = 4, 2048, 1024
H = 16
D = 64
G = 512            # feature columns per core (8 heads)
NH = 8             # heads per core
KC = 8             # contraction chunks of 128 over F
ST = 4             # s tiles of 512
TC = 16            # t chunks of 128
TP = 8             # t-chunk PAIRS (DoubleRow contracts 2 chunks at once)
SCALE = 1.0 / math.sqrt(S / H)
# exp is computed shifted by C_SHIFT: p' = exp(SCALE*l - C_SHIFT) so p' fits
# fp8e5m2 (max scaled logit 11.59 -> e^10.34 = 2^14.9 < 57344) AND the
# uint8-Schraudolph value below stays in [0, 124): no NaN encodings, and
# negative values (p' ~ 0) saturate to uint8 0 = +0.0. Numerator and
# denominator share the shift so the softmax ratio is exact.
C_SHIFT = 1.25
# Schraudolph exp2-bit-trick for DIRECT fp8e5m2 output on VectorE:
# round(l*SM + SB) as uint8 reinterpreted as e5m2 approximates
# exp(SCALE*l - C_SHIFT). 2^2 scales into the e5m2 exponent field;
# c=0.0579 centers the sawtooth (~2% rms, at the e5m2 quantization floor).
SCH_C = 0.0579
SCH_M = 4.0 * math.log2(math.e) * SCALE
SCH_B = 4.0 * (15.0 - SCH_C) - 4.0 * math.log2(math.e) * C_SHIFT
# Per-pair exp engine split: ScalarE spline exp (fp8 out) for 10 chunks,
# VectorE Schraudolph for 6; DVE chunks sit late in each pair-loop so the
# previous head pair's deferred epilogue (at pairs 0..1) never delays an
# exp the PE is about to consume.
DVE_T = frozenset((5, 7, 9, 11, 13, 15))
VD = 80            # padded per-head stride in vS8 (Ko step 640B, 16B-aligned)

_cached = None


def _build():
    import concourse.bacc as bacc
    import concourse.tile as tile
    from concourse import mybir
    from concourse.masks import make_identity

    dt = mybir.dt
    AF = mybir.ActivationFunctionType

    nc = bacc.Bacc("TRN2", target_bir_lowering=False, debug=False, num_devices=8)

    xt_d = nc.dram_tensor("xt", [128, KC, S], dt.bfloat16, kind="ExternalInput").ap()
    # Weights are host-packed [128, KC, G] (partition-major k-chunks) so each
    # matrix is ONE DMA: dma_start triggers cost ~650ns each on the issuing
    # engine, and 24 weight triggers were what delayed wv (and V-proj) by
    # ~14us at startup.
    wq_d = nc.dram_tensor("wq", [128, KC, G], dt.bfloat16, kind="ExternalInput").ap()
    wk_d = nc.dram_tensor("wk", [128, KC, G], dt.bfloat16, kind="ExternalInput").ap()
    wv_d = nc.dram_tensor("wv", [128, KC, G], dt.bfloat16, kind="ExternalInput").ap()
    res_d = nc.dram_tensor("res", [S, G], dt.bfloat16, kind="ExternalInput").ap()
    out_d = nc.dram_tensor("out", [S, G], dt.bfloat16, kind="ExternalOutput").ap()

    with tile.TileContext(nc) as tc:
        with tc.tile_pool(name="persist", bufs=1) as persist:
            identB = persist.tile([128, 128], dt.bfloat16)
            ones = persist.tile([128, NH], dt.float32)
            nbias = persist.tile([128, 1], dt.float32)
            scr = persist.tile([128, NH], dt.float32)
            xTall = persist.tile([128, KC, S], dt.bfloat16, name="xT")
            xT = [xTall[:, k, :] for k in range(KC)]
            kT = [persist.tile([128, S], dt.bfloat16, name=f"kT{m}") for m in range(4)]
            # fp8 V in DoubleRow pair layout: vS8[p][:, r, h, 0:D] holds V of
            # t-chunk 2p+r; col D is the all-ones softmax-denominator column.
            vS8 = [persist.tile([128, 2, NH, VD], dt.float8e4, name=f"vS8{p}")
                   for p in range(TP)]

            w_sb = {}
            with tc.tile_pool(name="wp", bufs=1) as wp:
                # DMA order is the consumption order of the prologue:
                #   sync:   xT(b0), wq, xT(b2), res...
                #   gpsimd: wk, xT(b1), xT(b3), wv
                # so K-proj(0,b) / Q-proj / V-proj each find their operands
                # just in time and the PE never drains waiting on X.
                def dma_w(nm, wd, eng):
                    t = wp.tile([128, KC, G], dt.bfloat16, name=f"w{nm}")
                    eng.dma_start(t[:], wd[:])
                    for k in range(KC):
                        w_sb[nm, k] = t[:, k]

                def dma_x(b, eng):
                    eng.dma_start(
                        xTall[:, :, b * 512:(b + 1) * 512],
                        xt_d[:, :, b * 512:(b + 1) * 512])

                dma_w("k", wk_d, nc.gpsimd)
                dma_x(0, nc.sync)
                dma_x(1, nc.gpsimd)
                dma_w("q", wq_d, nc.sync)
                dma_w("v", wv_d, nc.gpsimd)
                dma_x(2, nc.sync)
                dma_x(3, nc.gpsimd)

                # Init work AFTER the DMA triggers so the input transfers own
                # the HBM window from t=0; these run on otherwise-idle
                # engines while the DMAs stream.
                make_identity(nc, identB[:])
                nc.vector.memset(ones[:], 1.0)
                nc.vector.memset(nbias[:], -C_SHIFT)
                # Preload the exp table set on ScalarE while the DMAs run.
                nc.scalar.activation(scr[:], ones[:], AF.Exp)
                for p in range(TP):
                    nc.gpsimd.memset(vS8[p][:, :, :, D], 1.0)

                # PSUM: lp ring 3x2 banks (logits / proj accumulators / tp8
                # transpose scratch) + eps 2x1 banks = 8.
                with tc.tile_pool(name="pp", bufs=3, space="PSUM") as pp, \
                     tc.tile_pool(name="epp", bufs=2, space="PSUM") as epp, \
                     tc.tile_pool(name="qtsp", bufs=8) as qtsp, \
                     tc.tile_pool(name="expa", bufs=4) as expa, \
                     tc.tile_pool(name="esp", bufs=4) as esp, \
                     tc.tile_pool(name="stp", bufs=8) as stp, \
                     tc.tile_pool(name="rsp", bufs=3) as rsp, \
                     tc.tile_pool(name="rcp", bufs=4) as rcp:

                    def ptile():
                        return pp.tile([128, 1024], dt.float32, tag="lp", name="lp")

                    def emit_kproj(m, b):
                        pk = ptile()
                        for k in range(KC):
                            nc.tensor.matmul(
                                pk[:, 0:512], w_sb["k", k][:, m * 128:(m + 1) * 128],
                                xT[k][:, b * 512:(b + 1) * 512],
                                start=(k == 0), stop=(k == KC - 1))
                        nc.vector.tensor_copy(kT[m][:, b * 512:(b + 1) * 512],
                                              pk[:, 0:512])

                    def emit_vproj_j(b, j):
                        pv = ptile()
                        tci = b * 4 + j
                        for k in range(KC):
                            nc.tensor.matmul(
                                pv[:, 0:512],
                                xT[k][:, tci * 128:(tci + 1) * 128],
                                w_sb["v", k][:], start=(k == 0), stop=(k == KC - 1))
                        nc.vector.tensor_copy(
                            vS8[tci // 2][:, tci % 2, :, 0:D],
                            pv[:, 0:512].rearrange("p (h d) -> p h d", h=NH))

                    def emit_qproj(dst_st, m):
                        pq = ptile()
                        for k in range(KC):
                            nc.tensor.matmul(
                                pq[:, 0:512], w_sb["q", k][:, m * 128:(m + 1) * 128],
                                xT[k][:, dst_st * 512:(dst_st + 1) * 512],
                                start=(k == 0), stop=(k == KC - 1))
                        qt = qtsp.tile([128, 512], dt.bfloat16, tag="qts", name="qt")
                        nc.vector.tensor_copy(qt[:], pq[:, 0:512])
                        return qt

                    def emit_chunk(hp, qts, t, ex8, r):
                        # one 2-bank psum tile holds both halves' logits for
                        # this t-chunk: the two K=64 matmuls run concurrently
                        # in disjoint PE row groups. exp (shifted by C_SHIFT)
                        # lands in fp8 slot r of the pair tile ex8.
                        lp = ptile()
                        for half in range(2):
                            r0 = half * 64
                            nc.tensor.matmul(
                                lp[:, half * 512:(half + 1) * 512],
                                kT[hp][r0:r0 + 64, t * 128:(t + 1) * 128],
                                qts[r0:r0 + 64, :],
                                start=True, stop=True)
                        # exp: alternate engines so neither paces the loop.
                        if t in DVE_T:
                            nc.vector.tensor_scalar(
                                ex8[:, r, :].bitcast(dt.uint8), lp[:], SCH_M, SCH_B,
                                mybir.AluOpType.mult, mybir.AluOpType.add)
                        elif t < 4:
                            # pairs 0-1 have no DVE chunk; split these chunks
                            # by column across both engines so the serial
                            # ScalarE stretch (which gates the lp ring) is
                            # halved at every head-pair boundary.
                            nc.scalar.activation(ex8[:, r, 0:512], lp[:, 0:512],
                                                 AF.Exp, scale=SCALE,
                                                 bias=nbias[:])
                            nc.vector.tensor_scalar(
                                ex8[:, r, 512:1024].bitcast(dt.uint8),
                                lp[:, 512:1024], SCH_M, SCH_B,
                                mybir.AluOpType.mult, mybir.AluOpType.add)
                        else:
                            nc.scalar.activation(ex8[:, r, :], lp[:], AF.Exp,
                                                 scale=SCALE, bias=nbias[:])

                    def emit_pair(hp, qts, p):
                        ex8 = expa.tile([128, 2, 1024], dt.float8e5,
                                        tag="ex8", name="ex8")
                        emit_chunk(hp, qts, 2 * p, ex8, 0)
                        emit_chunk(hp, qts, 2 * p + 1, ex8, 1)
                        return ex8

                    # ---- PE warm-up spin: ~4.5us of dependency-free matmuls
                    # (identB @ identB) so the HAM activity window fills and
                    # the clock gate opens to 2.4 GHz BEFORE the real
                    # projections start. Without it the DMA-wait gaps in the
                    # prologue keep resetting the window and the first ~30us
                    # run at half clock. (Transpose-mode would not work here:
                    # it does not count as PE-busy for HAM.)
                    wu = ptile()
                    for _ in range(40):
                        nc.tensor.matmul(wu[:, 0:128], identB[:], identB[:],
                                         start=True, stop=True)

                    # ---- prologue, in DMA-arrival order: K-proj(0,b0) and
                    # Q-proj st=0 only need x-block b0 (+wk/wq, first on
                    # their queues); later K-proj blocks consume b1..b3 as
                    # they land; V-proj last (wv is the last weight DMA).
                    emit_kproj(0, 0)
                    qts_cur = [emit_qproj(0, m) for m in range(4)]
                    emit_kproj(0, 1)
                    emit_kproj(0, 2)
                    for j in range(4):
                        emit_vproj_j(0, j)
                    emit_kproj(0, 3)

                    # Remaining projections injected into the st=0 pair-loops,
                    # each emitted before its first consumer: vproj for t-chunks
                    # (2p+4, 2p+5) at pair-slot p (consumed at slot p+2 of the
                    # SAME hp=0 loop), kproj(m,*) anywhere before C(0,m) starts.
                    inject = {
                        (0, 0): dict(
                            [(p, [lambda b=b, j=j: emit_vproj_j(b, j)
                                  for (b, j) in (((2 * p + 4) // 4, (2 * p + 4) % 4),
                                                 ((2 * p + 5) // 4, (2 * p + 5) % 4))])
                             for p in range(6)] +
                            [(6, [lambda: emit_kproj(1, 0), lambda: emit_kproj(1, 1)]),
                             (7, [lambda: emit_kproj(1, 2), lambda: emit_kproj(1, 3)])]),
                        (0, 1): {1 + 2 * b: [lambda b=b: emit_kproj(2, b)]
                                 for b in range(4)},
                        (0, 2): {1 + 2 * b: [lambda b=b: emit_kproj(3, b)]
                                 for b in range(4)},
                    }

                    def make_epilogue(hp, ess, stage, fin=None, last=False):
                        # deferred epilogue tail (all VectorE, so cross-engine
                        # waits never sit at the head of ScalarE's exp FIFO):
                        # PE-transpose both halves into a psum-ring scratch
                        # (bitcast view), normalize by the ones-row
                        # denominator, write into stage. The effect^T bf16
                        # casts are NOT here -- they run at the owning
                        # iteration's end so the eps accumulator banks are
                        # free before the next iteration's effect(0). When
                        # this is hp=3's epilogue and the st-finish is
                        # pending, the residual-add + output DMA are
                        # interleaved j-major so the final DMAs start as
                        # early as possible.
                        def run():
                            tp8 = ptile()[:, 0:264].bitcast(dt.bfloat16) \
                                .rearrange("p (h c) -> p h c", c=D + 2)
                            for half in range(2):
                                for j in range(4):
                                    nc.tensor.transpose(
                                        tp8[:, half * 4 + j, 0:D + 1],
                                        ess[half][:, j * 128:(j + 1) * 128],
                                        identB[0:D + 1, 0:D + 1])
                            rec = rcp.tile([128, 8], dt.float32, tag="rec", name="rec")
                            nc.vector.reciprocal(rec[:], tp8[:, :, D])
                            for j in range(4):
                                for half in range(2):
                                    h = 2 * hp + half
                                    idx = half * 4 + j
                                    if last and half == 0:
                                        nc.scalar.activation(
                                            stage[j][:, h * 64:(h + 1) * 64],
                                            tp8[:, idx, 0:D], AF.Copy,
                                            scale=rec[:, idx:idx + 1])
                                    else:
                                        nc.vector.tensor_scalar_mul(
                                            stage[j][:, h * 64:(h + 1) * 64],
                                            tp8[:, idx, 0:D],
                                            rec[:, idx:idx + 1])
                                if fin is not None:
                                    s0, rts = fin
                                    nc.vector.tensor_add(stage[j][:], stage[j][:],
                                                         rts[j][:])
                                    nc.sync.dma_start(
                                        out_d[s0 + j * 128:s0 + (j + 1) * 128, :],
                                        stage[j][:])
                        return run

                    # The per-head-pair epilogue (and, for hp=3, the per-st
                    # residual+DMA) is deferred into the NEXT iteration's
                    # t-loop (t=0) so iteration boundaries never serialize
                    # the logits->exp->effect pipeline.
                    pend_epi = None
                    qts_next = [None] * 4
                    for st in range(ST):
                        s0 = st * 512
                        # Prefetch this tile's residual rows early (one DMA).
                        rtt = rsp.tile([128, 4, G], dt.bfloat16, tag="res", name="rt")
                        nc.sync.dma_start(
                            rtt[:], res_d[s0:s0 + 512, :]
                            .rearrange("(j p) g -> p j g", p=128))
                        rts = [rtt[:, j, :] for j in range(4)]
                        stage = [stp.tile([128, G], dt.bfloat16, tag="stage", name="stage")
                                 for _ in range(4)]
                        for hp in range(4):
                            inj = inject.get((st, hp), {})
                            eps = [epp.tile([D + 1, 512], dt.float32, tag="ep", name="ep")
                                   for _ in range(2)]
                            # software pipeline depth 2 pairs: logits(p+2) is
                            # emitted before effect(p) so the PE always has
                            # queued work to cover the exp latency.
                            exq = [emit_pair(hp, qts_cur[hp], 0),
                                   emit_pair(hp, qts_cur[hp], 1)]
                            for p in range(TP):
                                for fn in inj.get(p, ()):
                                    fn()
                                if p < TP - 2:
                                    exq.append(emit_pair(hp, qts_cur[hp], p + 2))
                                if p == 0 and pend_epi is not None:
                                    pend_epi()
                                    pend_epi = None
                                if p == 4 and st < ST - 1:
                                    qts_next[hp] = emit_qproj(st + 1, hp)
                                ex8 = exq.pop(0)
                                # effect^T: one DoubleRow matmul per head
                                # contracts BOTH t-chunks of the pair (2 fp8
                                # weights per PE cell, K=256 virtual).
                                for half in range(2):
                                    nc.tensor.matmul(
                                        eps[half][:],
                                        vS8[p][:, :, 2 * hp + half, 0:D + 1],
                                        ex8[:, :, half * 512:(half + 1) * 512],
                                        start=(p == 0), stop=(p == TP - 1),
                                        perf_mode=mybir.MatmulPerfMode.DoubleRow)
                            # cast effect^T to bf16 NOW (frees the eps psum
                            # banks for the next iteration's effect(0)); the
                            # last iteration splits the casts across ScalarE
                            # and VectorE since nothing else is in flight.
                            last = st == ST - 1 and hp == 3
                            ess = []
                            for half in range(2):
                                es = esp.tile([D + 1, 512], dt.bfloat16,
                                              tag="es", name="es")
                                if last and half == 0:
                                    nc.scalar.copy(es[:], eps[half][:])
                                else:
                                    nc.vector.tensor_copy(es[:], eps[half][:])
                                ess.append(es)
                            pend_epi = make_epilogue(
                                hp, ess, stage,
                                fin=(s0, rts) if hp == 3 else None,
                                last=last)
                        qts_cur = qts_next
                        qts_next = [None] * 4
                    pend_epi()

    nc.compile()
    return nc


def _get_nc():
    global _cached
    if _cached is None:
        _cached = _build()
    return _cached


def _make_in_maps(X, residual_score, wq, wk, wv):
    import ml_dtypes

    bf16 = ml_dtypes.bfloat16

    def pack(a):
        # [F, cols] -> [128, KC, cols] partition-major k-chunks (one DMA).
        return np.ascontiguousarray(
            a.reshape(KC, 128, a.shape[1]).transpose(1, 0, 2))

    X = np.asarray(X, dtype=np.float32)
    residual_score = np.asarray(residual_score, dtype=np.float32).astype(bf16)
    wq = np.asarray(wq, dtype=np.float32).astype(bf16)
    wk = np.asarray(wk, dtype=np.float32).astype(bf16)
    wv = np.asarray(wv, dtype=np.float32).astype(bf16)
    xts = [pack(X[b].T.astype(bf16)) for b in range(B)]
    in_maps = []
    for c in range(8):
        b, g = c // 2, c % 2
        cols = slice(g * G, (g + 1) * G)
        in_maps.append({
            "xt": xts[b],
            "wq": pack(wq[:, cols]),
            "wk": pack(wk[:, cols]),
            "wv": pack(wv[:, cols]),
            "res": np.ascontiguousarray(residual_score[b, :, cols]),
        })
    return in_maps


def _assemble(results):
    out = np.empty((B, S, F), dtype=np.float32)
    for c in range(8):
        b, g = c // 2, c % 2
        out[b, :, g * G:(g + 1) * G] = np.asarray(results[c]["out"],
                                                  dtype=np.float32)
    return out


def run(X, residual_score, wq, wk, wv, trace=False):
    from concourse.bass_utils import run_bass_kernel_spmd

    nc = _get_nc()
    in_maps = _make_in_maps(X, residual_score, wq, wk, wv)
    res = run_bass_kernel_spmd(nc, in_maps, core_ids=list(range(8)), trace=trace)
    return _assemble(res.results), res


def kernel(X, residual_score, wq, wk, wv):
    out, _ = run(X, residual_score, wq, wk, wv)
    return (out, out)

